# revision 18
# baseline (speedup 1.0000x reference)
"""Trainium2 Bass kernel for nn_CoordsToNRF.

out[b, p] = atom_nc[b, p] * (AU2KCALMOLA / MAX_NRF) / ||coords[b, I[p]] - coords[b, J[p]]||^2

Strategy (pure data parallel over batch, 8 cores x 128 batches):
  - Layout: batch on partitions, pairs on the free dim.
  - Pair gather+subtract on the TensorEngine: per xyz component,
        D_c = CT_c.T @ S
    with S [atom, pairs] the +1/-1 tril selection matrix. Exact TWO-term
    fp16 split (~22 mantissa bits):  C = C0 + 2^-14*C1.
    The 2^-14 for the lo term is folded into a GLOBAL rescale so one S
    matrix serves both terms with no extra DMA or engine work:
        S' = S * 2^-11        (+-2^-11, exact fp16)
        CT_hi = C0 * 2^11     (exact exponent shift, |C0|*2^11 < 60000)
        CT_lo = C1 * 2^-3     (exact; subnormal flush loses < 3e-8)
    so  CT_hi.T @ S' + CT_lo.T @ S' = C0.T@S + 2^-14 * C1.T@S = D exactly.
  - Per 512-col group (one 3-bank PSUM tile): one ScalarE Square op over
    the 3 planes (scale folds 1/sqrt(K)), bf16 out.
  - Per 1024-col macro: DVE-only tail (adds at the bf16 2x rate, fast
    reciprocal, mul by fp16 atom_nc). GpSimd stays IDLE on purpose: the
    hardware activity limiter duty-caps the DVE+GpSimd pair at 50%, so
    any GpSimd work throttles the DVE ~3x (measured).
  - Head: input DMAs split across BOTH HWDGE rings -- ct then anc on the
    ScalarE ring, smat slices (small first) + outputs on the Sync ring.
    ct and smat[0:512] land concurrently ~9.5us (vs ~12us serialized).
    Matmuls run hi-term-first; dummy-matmul warmup bridges preamble ->
    first data so the PE HAM un-throttles by ~10us.
  - Tail: the last macro drains as one 512 chain plus two 256 chains
    (half-width final squares), keeping the after-last-matmul critical
    path short.
  - DMA halved vs f32: atom_nc uploaded fp16, output returned bf16.
"""

import sys

for _p in ("/opt/trn_rl_repo",):
    if _p not in sys.path:
        sys.path.insert(0, _p)

import numpy as np
import ml_dtypes
from contextlib import ExitStack

import concourse.bass as bass
import concourse.bacc as bacc
import concourse.tile as tile
from concourse import mybir
from concourse.bass_utils import run_bass_kernel_spmd
from concourse.dve_ops import RECIP_APPROX_FAST_CONSTS, RECIPROCAL_APPROX_FAST

F32 = mybir.dt.float32
F16 = mybir.dt.float16
BF16 = mybir.dt.bfloat16

N_ATOMS = 128
NC2 = N_ATOMS * (N_ATOMS - 1) // 2  # 8128
NPAD = 8192  # pad pairs to uniform 512-col groups; host drops the tail
BATCH = 1024
N_CORES = 8
BPC = BATCH // N_CORES  # 128 batches per core

AU2KCALMOLA = 627.5095 * 0.529177
MAX_NRF = 13036.0
K_CONST = AU2KCALMOLA / MAX_NRF
SQ_SCALE = float(1.0 / np.sqrt(K_CONST))  # fold K into the square
LO_SHIFT = 2.0 ** 14
HI_UP = 2.0 ** 11    # ct_hi scale (exact shift)
LO_DN = 2.0 ** -3    # ct_lo scale: 2^11 * 2^-14
S_DN = 2.0 ** -11    # smat scale

GROUP = 512           # one 3-bank PSUM tile per group
NG = NPAD // GROUP    # 16 groups
MACRO = 1024          # elementwise unit = 2 groups
NMAC = NPAD // MACRO  # 8 macros

# smat DMA slices (Sync HWDGE ring): small first slices so early groups
# start as soon as possible; ct/anc ride the ScalarE HWDGE ring in
# parallel. Keep the total DMA count moderate: only ~8 DMAHW semaphore
# lanes exist, and a DMA whose lane is still busy stalls its whole
# queue (16 small slices starved the PE for 6.9us).
SMAT_SLICES = [(0, 512), (512, 512), (1024, 1024), (2048, 2048),
               (4096, 2048), (6144, 2048)]
ANC_SLICES = [(0, 1024), (1024, 2048), (3072, 2048), (5120, 3072)]
# Dummy-matmul warmup: bridges the idle window between engine-preamble
# end (~7.2us) and the first smat slice landing so the PE p-state ramp
# is continuous and early groups run at full clock.
N_WARMUP_MM = 6

_I, _J = np.tril_indices(N_ATOMS, -1)


def _build_smat() -> np.ndarray:
    s = np.zeros((N_ATOMS, NPAD), dtype=np.float16)
    p = np.arange(NC2)
    s[_I, p] = S_DN
    s[_J, p] = -S_DN
    return s


def _build_program():
    nc = bacc.Bacc("TRN2", target_bir_lowering=False, debug=False)

    # ct: [atom, term, comp, batch] fp16, pre-transposed/split/scaled on host
    ct_d = nc.dram_tensor("ct", [N_ATOMS, 2 * 3 * BPC], F16, kind="ExternalInput")
    anc_d = nc.dram_tensor("anc", [BPC, NPAD], F16, kind="ExternalInput")
    smat_d = nc.dram_tensor("smat", [N_ATOMS, NPAD], F16, kind="ExternalInput")
    out_d = nc.dram_tensor("out", [BPC, NPAD], BF16, kind="ExternalOutput")

    rc = RECIP_APPROX_FAST_CONSTS

    with tile.TileContext(nc) as tc, ExitStack() as ctx:
        const = ctx.enter_context(tc.tile_pool(name="const", bufs=1))
        sqp = ctx.enter_context(tc.tile_pool(name="sqp", bufs=3))
        work = ctx.enter_context(tc.tile_pool(name="work", bufs=3))
        outp = ctx.enter_context(tc.tile_pool(name="outp", bufs=3))
        ps = ctx.enter_context(tc.tile_pool(name="ps", bufs=2, space="PSUM"))

        # ---- inputs ----
        # ct + anc on the ScalarE HWDGE ring (ScalarE is idle until the
        # first square); smat slices + outputs on the Sync ring. The two
        # rings transfer concurrently, halving the input-latency chain.
        ct_sb = const.tile([N_ATOMS, 2, 3, BPC], F16)
        nc.scalar.dma_start(
            ct_sb[:], ct_d[:, :].rearrange("a (t c b) -> a t c b", t=2, c=3)
        )
        smat_sb = {}
        for s0, w in SMAT_SLICES:
            st = const.tile([N_ATOMS, w], F16, tag=f"smat{s0}")
            nc.sync.dma_start(st[:], smat_d[:, s0:s0 + w])
            for g0 in range(s0, s0 + w, GROUP):
                smat_sb[g0] = (st, g0 - s0)
        anc_sb = []
        for s0, w in ANC_SLICES:
            at = const.tile([BPC, w], F16, tag=f"anc{s0}")
            nc.scalar.dma_start(at[:], anc_d[:, s0:s0 + w])
            anc_sb.append((s0, w, at))

        def anc_ap(c0, w):
            for s0, sw, at in anc_sb:
                if s0 <= c0 and c0 + w <= s0 + sw:
                    return at[:, c0 - s0:c0 - s0 + w]
            raise KeyError(c0)

        # ---- PE warmup. The HAM un-throttles the PE clock only after
        # ~3.4us of sustained activity, so start immediately after the
        # preamble barrier: a few tiny N=1 matmuls on the framework's
        # pre-initialized const tensors (no memset dependency), then
        # full-width dummy matmuls once warm_sb's memset lands. Sized so
        # warmup ends right as the first real smat/ct data arrives.
        warm_sb = const.tile([BPC, GROUP], F16, tag="warm")
        nc.gpsimd.memset(warm_sb[:], 0.0)
        warm_ps = ps.tile([BPC, GROUP], F32, tag="warm_ps")
        czero = nc.const_aps.tensor(0.0, [BPC, 1], F32)
        for _ in range(4):
            nc.tensor.matmul(
                warm_ps[0:1, 0:1], czero, czero, start=True, stop=True,
            )
        for _ in range(N_WARMUP_MM):
            nc.tensor.matmul(
                warm_ps[:, :], warm_sb[:, 0:BPC], warm_sb[:, :],
                start=True, stop=True,
            )

        def mm_group(g0, d_t):
            st, off = smat_sb[g0]
            for t in range(2):
                for c in range(3):
                    nc.tensor.matmul(
                        d_t[:, c, :],
                        ct_sb[:, t, c, :],
                        st[:, off:off + GROUP],
                        start=(t == 0),
                        stop=(t == 1),
                    )

        def recip(out_ap, in_ap):
            nc.vector._custom_dve(
                RECIPROCAL_APPROX_FAST, out=out_ap, in0=in_ap,
                s0=rc["s0"], s1=rc["s1"], imm2=rc["imm2"],
            )

        def square_group(sq_t, gi, d_t):
            # all 3 planes in one ScalarE op (flat AP saves ~110ns/op)
            nc.scalar.activation(
                sq_t[:, gi, :, :].rearrange("b c w -> b (c w)"),
                d_t[:, :, :].rearrange("b c w -> b (c w)"),
                mybir.ActivationFunctionType.Square,
                bias=0.0, scale=SQ_SCALE,
            )

        # ---- macros 0..6: steady-state pipeline ----
        for m in range(NMAC - 1):
            sq_t = sqp.tile([BPC, 2, 3, GROUP], BF16, tag="sq")
            for gi in range(2):
                g0 = m * MACRO + gi * GROUP
                d_t = ps.tile([BPC, 3, GROUP], F32, tag="d")
                mm_group(g0, d_t)
                square_group(sq_t, gi, d_t)
            # All-bf16 DVE tail: adds on the 2x path, fast reciprocal,
            # multiply by fp16 atom_nc.
            t01 = work.tile([BPC, MACRO], BF16, tag="t01")
            t01v = t01[:, :].rearrange("b (g w) -> b g w", g=2)
            nc.vector.tensor_add(t01v, sq_t[:, :, 0, :], sq_t[:, :, 1, :])
            r2 = work.tile([BPC, MACRO], BF16, tag="r2")
            r2v = r2[:, :].rearrange("b (g w) -> b g w", g=2)
            nc.vector.tensor_add(r2v, t01v, sq_t[:, :, 2, :])
            inv = work.tile([BPC, MACRO], BF16, tag="inv")
            # reciprocal_approx_fast with bf16 in/out APs: the DVE read
            # stage converts bf16->f32 lanes exactly (bits<<16), so the
            # bitwise seed trick is unaffected. Validated on HW: 3e-6.
            recip(inv[:, :], r2[:, :])
            o = outp.tile([BPC, MACRO], BF16)
            c0 = m * MACRO
            nc.vector.tensor_mul(o[:, :], inv[:, :], anc_ap(c0, MACRO))
            nc.sync.dma_start(out_d[:, c0:c0 + MACRO], o[:, :])

        # ---- macro 7: short-chain drain ----
        # Group 14's 512 cols drain as one chain while group 15's matmuls
        # still run. Group 15 squares as two half-width ACTIVATEs feeding
        # two 256-col chains, so the after-last-matmul path is
        # sq(256) + short DVE chain + small DMA.
        sq_t = sqp.tile([BPC, 2, 3, GROUP], BF16, tag="sq")
        g14 = (NMAC - 1) * MACRO
        d14 = ps.tile([BPC, 3, GROUP], F32, tag="d")
        mm_group(g14, d14)
        nc.scalar.activation(
            sq_t[:, 0, :, :].rearrange("b c w -> b (c w)"),
            d14[:, :, :].rearrange("b c w -> b (c w)"),
            mybir.ActivationFunctionType.Square, bias=0.0, scale=SQ_SCALE,
        )
        g15 = g14 + GROUP
        d15 = ps.tile([BPC, 3, GROUP], F32, tag="d")
        mm_group(g15, d15)

        # chain A: group 14's 512 cols
        t01 = work.tile([BPC, MACRO], BF16, tag="t01")
        r2 = work.tile([BPC, MACRO], BF16, tag="r2")
        inv = work.tile([BPC, MACRO], BF16, tag="inv")
        o = outp.tile([BPC, MACRO], BF16)
        nc.vector.tensor_add(t01[:, 0:GROUP], sq_t[:, 0, 0, :], sq_t[:, 0, 1, :])
        nc.vector.tensor_add(r2[:, 0:GROUP], t01[:, 0:GROUP], sq_t[:, 0, 2, :])
        recip(inv[:, 0:GROUP], r2[:, 0:GROUP])
        nc.vector.tensor_mul(o[:, 0:GROUP], inv[:, 0:GROUP], anc_ap(g14, GROUP))
        nc.sync.dma_start(out_d[:, g14:g14 + GROUP], o[:, 0:GROUP])

        # group 15: two half-width square+chain pipelines
        H = GROUP // 2
        for h in range(2):
            hs = slice(h * H, (h + 1) * H)
            nc.scalar.activation(
                sq_t[:, 1, :, hs], d15[:, :, hs],
                mybir.ActivationFunctionType.Square, bias=0.0, scale=SQ_SCALE,
            )
            lo = GROUP + h * H
            c0 = g14 + lo
            nc.vector.tensor_add(
                t01[:, lo:lo + H], sq_t[:, 1, 0, hs], sq_t[:, 1, 1, hs],
            )
            nc.vector.tensor_add(
                r2[:, lo:lo + H], t01[:, lo:lo + H], sq_t[:, 1, 2, hs],
            )
            recip(inv[:, lo:lo + H], r2[:, lo:lo + H])
            nc.vector.tensor_mul(
                o[:, lo:lo + H], inv[:, lo:lo + H], anc_ap(c0, H)
            )
            nc.sync.dma_start(out_d[:, c0:c0 + H], o[:, lo:lo + H])

    nc.compile()
    return nc


_CACHED = None


def _get_program():
    global _CACHED
    if _CACHED is None:
        _CACHED = _build_program()
    return _CACHED


def _host_prep(coords, atom_nc):
    """Host-side sharding/layout only: fp16 hi/lo split (with the exact
    2^11 / 2^-3 exponent-shift scaling), transpose to [atom, term, comp,
    batch], fp16 atom_nc, padding to NPAD."""
    c32 = coords.astype(np.float32)
    c0 = c32.astype(np.float16)
    c1 = ((c32.astype(np.float64) - c0.astype(np.float64)) * LO_SHIFT).astype(
        np.float16
    )
    assert np.abs(c0.astype(np.float32)).max() * HI_UP < 60000.0
    hi = (c0.astype(np.float32) * HI_UP).astype(np.float16)
    lo = (c1.astype(np.float32) * LO_DN).astype(np.float16)
    # [B, A, 3] -> [cores, atom, term, comp, bpc]
    ct = np.empty((N_CORES, N_ATOMS, 2, 3, BPC), dtype=np.float16)
    for t, cc in enumerate((hi, lo)):
        r = cc.reshape(N_CORES, BPC, N_ATOMS, 3)
        ct[:, :, t, :, :] = r.transpose(0, 2, 3, 1)
    anc16 = np.ones((BATCH, NPAD), dtype=np.float16)
    anc16[:, :NC2] = atom_nc.astype(np.float16)
    return ct, anc16


def kernel(coords, atom_nc, _trace=False, _trace_kwargs=None):
    coords = np.ascontiguousarray(np.asarray(coords, dtype=np.float32))
    atom_nc = np.ascontiguousarray(np.asarray(atom_nc, dtype=np.float32))
    assert coords.shape == (BATCH, N_ATOMS, 3)
    assert atom_nc.shape == (BATCH, NC2)

    nc = _get_program()
    smat = _build_smat()
    ct, anc16 = _host_prep(coords, atom_nc)

    in_maps = []
    for core in range(N_CORES):
        b0 = core * BPC
        in_maps.append({
            "ct": ct[core].reshape(N_ATOMS, 2 * 3 * BPC),
            "anc": anc16[b0:b0 + BPC],
            "smat": smat,
        })

    kw = {}
    if _trace:
        kw["trace"] = True
        kw.update(_trace_kwargs or {})
    res = run_bass_kernel_spmd(nc, in_maps, core_ids=list(range(N_CORES)), **kw)
    out = np.concatenate(
        [r["out"][:, :NC2].astype(np.float32) for r in res.results], axis=0
    )
    if _trace:
        return out, res
    return out


if __name__ == "__main__":
    rng = np.random.default_rng(0)
    coords = (rng.standard_normal((BATCH, N_ATOMS, 3)) * 5.0).astype(np.float32)
    atom_nc = rng.uniform(1.0, 50.0, (BATCH, NC2)).astype(np.float32)
    out = kernel(coords, atom_nc)
    print(out.shape, out.dtype)


# revision 19
# speedup vs baseline: 1.0138x; 1.0138x over previous
"""Trainium2 Bass kernel for nn_CoordsToNRF.

out[b, p] = atom_nc[b, p] * (AU2KCALMOLA / MAX_NRF) / ||coords[b, I[p]] - coords[b, J[p]]||^2

Strategy (pure data parallel over batch, 8 cores x 128 batches):
  - Layout: batch on partitions, pairs on the free dim.
  - Pair gather+subtract on the TensorEngine: per xyz component,
        D_c = CT_c.T @ S
    with S [atom, pairs] the +1/-1 tril selection matrix. Exact TWO-term
    fp16 split (~22 mantissa bits):  C = C0 + 2^-14*C1.
    The 2^-14 for the lo term is folded into a GLOBAL rescale so one S
    matrix serves both terms with no extra DMA or engine work:
        S' = S * 2^-11        (+-2^-11, exact fp16)
        CT_hi = C0 * 2^11     (exact exponent shift, |C0|*2^11 < 60000)
        CT_lo = C1 * 2^-3     (exact; subnormal flush loses < 3e-8)
    so  CT_hi.T @ S' + CT_lo.T @ S' = C0.T@S + 2^-14 * C1.T@S = D exactly.
  - Per 512-col group (one 3-bank PSUM tile): one ScalarE Square op over
    the 3 planes (scale folds 1/sqrt(K)), bf16 out.
  - Per 1024-col macro: DVE-only tail (adds at the bf16 2x rate, fast
    reciprocal, mul by fp16 atom_nc). GpSimd stays IDLE on purpose: the
    hardware activity limiter duty-caps the DVE+GpSimd pair at 50%, so
    any GpSimd work throttles the DVE ~3x (measured).
  - Head: input DMAs split across BOTH HWDGE rings -- ct then anc on the
    ScalarE ring, smat slices (small first) + outputs on the Sync ring.
    ct and smat[0:512] land concurrently ~9.5us (vs ~12us serialized).
    Matmuls run hi-term-first; dummy-matmul warmup bridges preamble ->
    first data so the PE HAM un-throttles by ~10us.
  - Tail: the last macro drains as one 512 chain plus two 256 chains
    (half-width final squares), keeping the after-last-matmul critical
    path short.
  - DMA halved vs f32: atom_nc uploaded fp16, output returned bf16.
"""

import sys

for _p in ("/opt/trn_rl_repo",):
    if _p not in sys.path:
        sys.path.insert(0, _p)

import numpy as np
import ml_dtypes
from contextlib import ExitStack

import concourse.bass as bass
import concourse.bacc as bacc
import concourse.tile as tile
from concourse import mybir
from concourse.bass_utils import run_bass_kernel_spmd
from concourse.dve_ops import RECIP_APPROX_FAST_CONSTS, RECIPROCAL_APPROX_FAST

F32 = mybir.dt.float32
F16 = mybir.dt.float16
BF16 = mybir.dt.bfloat16

N_ATOMS = 128
NC2 = N_ATOMS * (N_ATOMS - 1) // 2  # 8128
NPAD = 8192  # pad pairs to uniform 512-col groups; host drops the tail
BATCH = 1024
N_CORES = 8
BPC = BATCH // N_CORES  # 128 batches per core

AU2KCALMOLA = 627.5095 * 0.529177
MAX_NRF = 13036.0
K_CONST = AU2KCALMOLA / MAX_NRF
SQ_SCALE = float(1.0 / np.sqrt(K_CONST))  # fold K into the square
LO_SHIFT = 2.0 ** 14
HI_UP = 2.0 ** 11    # ct_hi scale (exact shift)
LO_DN = 2.0 ** -3    # ct_lo scale: 2^11 * 2^-14
S_DN = 2.0 ** -11    # smat scale

GROUP = 512           # one 3-bank PSUM tile per group
NG = NPAD // GROUP    # 16 groups
MACRO = 1024          # elementwise unit = 2 groups
NMAC = NPAD // MACRO  # 8 macros

# smat DMA slices (Sync HWDGE ring): small first slices so early groups
# start as soon as possible; ct/anc ride the ScalarE HWDGE ring in
# parallel. Keep the total DMA count moderate: only ~8 DMAHW semaphore
# lanes exist, and a DMA whose lane is still busy stalls its whole
# queue (16 small slices starved the PE for 6.9us).
SMAT_SLICES = [(0, 512), (512, 512), (1024, 1024), (2048, 2048),
               (4096, 2048), (6144, 2048)]
ANC_SLICES = [(0, 1024), (1024, 2048), (3072, 2048), (5120, 3072)]
# Dummy-matmul warmup: bridges the idle window between engine-preamble
# end (~7.2us) and the first smat slice landing so the PE p-state ramp
# is continuous and early groups run at full clock.
N_WARMUP_MM = 6

_I, _J = np.tril_indices(N_ATOMS, -1)


def _build_smat() -> np.ndarray:
    s = np.zeros((N_ATOMS, NPAD), dtype=np.float16)
    p = np.arange(NC2)
    s[_I, p] = S_DN
    s[_J, p] = -S_DN
    return s


def _build_program():
    nc = bacc.Bacc("TRN2", target_bir_lowering=False, debug=False)

    # ct: [atom, term, comp, batch] fp16, pre-transposed/split/scaled on host
    ct_d = nc.dram_tensor("ct", [N_ATOMS, 2 * 3 * BPC], F16, kind="ExternalInput")
    anc_d = nc.dram_tensor("anc", [BPC, NPAD], F16, kind="ExternalInput")
    smat_d = nc.dram_tensor("smat", [N_ATOMS, NPAD], F16, kind="ExternalInput")
    out_d = nc.dram_tensor("out", [BPC, NPAD], BF16, kind="ExternalOutput")

    rc = RECIP_APPROX_FAST_CONSTS

    with tile.TileContext(nc) as tc, ExitStack() as ctx:
        const = ctx.enter_context(tc.tile_pool(name="const", bufs=1))
        sqp = ctx.enter_context(tc.tile_pool(name="sqp", bufs=3))
        work = ctx.enter_context(tc.tile_pool(name="work", bufs=3))
        outp = ctx.enter_context(tc.tile_pool(name="outp", bufs=3))
        ps = ctx.enter_context(tc.tile_pool(name="ps", bufs=2, space="PSUM"))

        # ---- inputs ----
        # ct + anc on the ScalarE HWDGE ring (ScalarE is idle until the
        # first square); smat slices + outputs on the Sync ring. The two
        # rings transfer concurrently, halving the input-latency chain.
        ct_sb = const.tile([N_ATOMS, 2, 3, BPC], F16)
        nc.scalar.dma_start(
            ct_sb[:], ct_d[:, :].rearrange("a (t c b) -> a t c b", t=2, c=3)
        )
        smat_sb = {}
        for s0, w in SMAT_SLICES:
            st = const.tile([N_ATOMS, w], F16, tag=f"smat{s0}")
            nc.sync.dma_start(st[:], smat_d[:, s0:s0 + w])
            for g0 in range(s0, s0 + w, GROUP):
                smat_sb[g0] = (st, g0 - s0)
        anc_sb = []
        for s0, w in ANC_SLICES:
            at = const.tile([BPC, w], F16, tag=f"anc{s0}")
            nc.scalar.dma_start(at[:], anc_d[:, s0:s0 + w])
            anc_sb.append((s0, w, at))

        def anc_ap(c0, w):
            for s0, sw, at in anc_sb:
                if s0 <= c0 and c0 + w <= s0 + sw:
                    return at[:, c0 - s0:c0 - s0 + w]
            raise KeyError(c0)

        # ---- PE warmup. The HAM un-throttles the PE clock only after
        # ~3.4us of sustained activity, so start immediately after the
        # preamble barrier: a few tiny N=1 matmuls on the framework's
        # pre-initialized const tensors (no memset dependency), then
        # full-width dummy matmuls once warm_sb's memset lands. Sized so
        # warmup ends right as the first real smat/ct data arrives.
        warm_sb = const.tile([BPC, GROUP], F16, tag="warm")
        nc.gpsimd.memset(warm_sb[:], 0.0)
        warm_ps = ps.tile([BPC, GROUP], F32, tag="warm_ps")
        czero = nc.const_aps.tensor(0.0, [BPC, 1], F32)
        for _ in range(4):
            nc.tensor.matmul(
                warm_ps[0:1, 0:1], czero, czero, start=True, stop=True,
            )
        for _ in range(N_WARMUP_MM):
            nc.tensor.matmul(
                warm_ps[:, :], warm_sb[:, 0:BPC], warm_sb[:, :],
                start=True, stop=True,
            )

        def mm_group(g0, d_t):
            st, off = smat_sb[g0]
            for t in range(2):
                for c in range(3):
                    nc.tensor.matmul(
                        d_t[:, c, :],
                        ct_sb[:, t, c, :],
                        st[:, off:off + GROUP],
                        start=(t == 0),
                        stop=(t == 1),
                    )

        def recip(out_ap, in_ap):
            nc.vector._custom_dve(
                RECIPROCAL_APPROX_FAST, out=out_ap, in0=in_ap,
                s0=rc["s0"], s1=rc["s1"], imm2=rc["imm2"],
            )

        def square_group(sq_t, gi, d_t):
            # all 3 planes in one ScalarE op (flat AP saves ~110ns/op)
            nc.scalar.activation(
                sq_t[:, gi, :, :].rearrange("b c w -> b (c w)"),
                d_t[:, :, :].rearrange("b c w -> b (c w)"),
                mybir.ActivationFunctionType.Square,
                bias=0.0, scale=SQ_SCALE,
            )

        # ---- macros 0..6: steady-state pipeline ----
        for m in range(NMAC - 1):
            sq_t = sqp.tile([BPC, 2, 3, GROUP], BF16, tag="sq")
            for gi in range(2):
                g0 = m * MACRO + gi * GROUP
                d_t = ps.tile([BPC, 3, GROUP], F32, tag="d")
                mm_group(g0, d_t)
                square_group(sq_t, gi, d_t)
            # All-bf16 DVE tail: adds on the 2x path, fast reciprocal,
            # multiply by fp16 atom_nc.
            t01 = work.tile([BPC, MACRO], BF16, tag="t01")
            t01v = t01[:, :].rearrange("b (g w) -> b g w", g=2)
            nc.vector.tensor_add(t01v, sq_t[:, :, 0, :], sq_t[:, :, 1, :])
            r2 = work.tile([BPC, MACRO], BF16, tag="r2")
            r2v = r2[:, :].rearrange("b (g w) -> b g w", g=2)
            nc.vector.tensor_add(r2v, t01v, sq_t[:, :, 2, :])
            inv = work.tile([BPC, MACRO], BF16, tag="inv")
            # reciprocal_approx_fast with bf16 in/out APs: the DVE read
            # stage converts bf16->f32 lanes exactly (bits<<16), so the
            # bitwise seed trick is unaffected. Validated on HW: 3e-6.
            recip(inv[:, :], r2[:, :])
            o = outp.tile([BPC, MACRO], BF16)
            c0 = m * MACRO
            nc.vector.tensor_mul(o[:, :], inv[:, :], anc_ap(c0, MACRO))
            nc.sync.dma_start(out_d[:, c0:c0 + MACRO], o[:, :])
            if m == 0:
                # Bridge warmups: during the pipeline ramp the PE stalls
                # 1.5-2.7us here waiting for the first squares to free
                # the PSUM ring -- close to the HAM's 3.4us re-throttle
                # window (the occasional catastrophic-slow run). Keep the
                # PE array busy through the stall with throwaway matmuls.
                for _ in range(3):
                    nc.tensor.matmul(
                        warm_ps[:, :], warm_sb[:, 0:BPC], warm_sb[:, :],
                        start=True, stop=True,
                    )

        # ---- macro 7: short-chain drain ----
        # Group 14's 512 cols drain as one chain while group 15's matmuls
        # still run. Group 15 squares as two half-width ACTIVATEs feeding
        # two 256-col chains, so the after-last-matmul path is
        # sq(256) + short DVE chain + small DMA.
        sq_t = sqp.tile([BPC, 2, 3, GROUP], BF16, tag="sq")
        g14 = (NMAC - 1) * MACRO
        d14 = ps.tile([BPC, 3, GROUP], F32, tag="d")
        mm_group(g14, d14)
        nc.scalar.activation(
            sq_t[:, 0, :, :].rearrange("b c w -> b (c w)"),
            d14[:, :, :].rearrange("b c w -> b (c w)"),
            mybir.ActivationFunctionType.Square, bias=0.0, scale=SQ_SCALE,
        )
        g15 = g14 + GROUP
        d15 = ps.tile([BPC, 3, GROUP], F32, tag="d")
        mm_group(g15, d15)

        # chain A: group 14's 512 cols
        t01 = work.tile([BPC, MACRO], BF16, tag="t01")
        r2 = work.tile([BPC, MACRO], BF16, tag="r2")
        inv = work.tile([BPC, MACRO], BF16, tag="inv")
        o = outp.tile([BPC, MACRO], BF16)
        nc.vector.tensor_add(t01[:, 0:GROUP], sq_t[:, 0, 0, :], sq_t[:, 0, 1, :])
        nc.vector.tensor_add(r2[:, 0:GROUP], t01[:, 0:GROUP], sq_t[:, 0, 2, :])
        recip(inv[:, 0:GROUP], r2[:, 0:GROUP])
        nc.vector.tensor_mul(o[:, 0:GROUP], inv[:, 0:GROUP], anc_ap(g14, GROUP))
        nc.sync.dma_start(out_d[:, g14:g14 + GROUP], o[:, 0:GROUP])

        # group 15: two half-width square+chain pipelines
        H = GROUP // 2
        for h in range(2):
            hs = slice(h * H, (h + 1) * H)
            nc.scalar.activation(
                sq_t[:, 1, :, hs], d15[:, :, hs],
                mybir.ActivationFunctionType.Square, bias=0.0, scale=SQ_SCALE,
            )
            lo = GROUP + h * H
            c0 = g14 + lo
            nc.vector.tensor_add(
                t01[:, lo:lo + H], sq_t[:, 1, 0, hs], sq_t[:, 1, 1, hs],
            )
            nc.vector.tensor_add(
                r2[:, lo:lo + H], t01[:, lo:lo + H], sq_t[:, 1, 2, hs],
            )
            recip(inv[:, lo:lo + H], r2[:, lo:lo + H])
            nc.vector.tensor_mul(
                o[:, lo:lo + H], inv[:, lo:lo + H], anc_ap(c0, H)
            )
            nc.sync.dma_start(out_d[:, c0:c0 + H], o[:, lo:lo + H])

    nc.compile()
    return nc


_CACHED = None


def _get_program():
    global _CACHED
    if _CACHED is None:
        _CACHED = _build_program()
    return _CACHED


def _host_prep(coords, atom_nc):
    """Host-side sharding/layout only: fp16 hi/lo split (with the exact
    2^11 / 2^-3 exponent-shift scaling), transpose to [atom, term, comp,
    batch], fp16 atom_nc, padding to NPAD."""
    c32 = coords.astype(np.float32)
    c0 = c32.astype(np.float16)
    c1 = ((c32.astype(np.float64) - c0.astype(np.float64)) * LO_SHIFT).astype(
        np.float16
    )
    assert np.abs(c0.astype(np.float32)).max() * HI_UP < 60000.0
    hi = (c0.astype(np.float32) * HI_UP).astype(np.float16)
    lo = (c1.astype(np.float32) * LO_DN).astype(np.float16)
    # [B, A, 3] -> [cores, atom, term, comp, bpc]
    ct = np.empty((N_CORES, N_ATOMS, 2, 3, BPC), dtype=np.float16)
    for t, cc in enumerate((hi, lo)):
        r = cc.reshape(N_CORES, BPC, N_ATOMS, 3)
        ct[:, :, t, :, :] = r.transpose(0, 2, 3, 1)
    anc16 = np.ones((BATCH, NPAD), dtype=np.float16)
    anc16[:, :NC2] = atom_nc.astype(np.float16)
    return ct, anc16


def kernel(coords, atom_nc, _trace=False, _trace_kwargs=None):
    coords = np.ascontiguousarray(np.asarray(coords, dtype=np.float32))
    atom_nc = np.ascontiguousarray(np.asarray(atom_nc, dtype=np.float32))
    assert coords.shape == (BATCH, N_ATOMS, 3)
    assert atom_nc.shape == (BATCH, NC2)

    nc = _get_program()
    smat = _build_smat()
    ct, anc16 = _host_prep(coords, atom_nc)

    in_maps = []
    for core in range(N_CORES):
        b0 = core * BPC
        in_maps.append({
            "ct": ct[core].reshape(N_ATOMS, 2 * 3 * BPC),
            "anc": anc16[b0:b0 + BPC],
            "smat": smat,
        })

    kw = {}
    if _trace:
        kw["trace"] = True
        kw.update(_trace_kwargs or {})
    res = run_bass_kernel_spmd(nc, in_maps, core_ids=list(range(N_CORES)), **kw)
    out = np.concatenate(
        [r["out"][:, :NC2].astype(np.float32) for r in res.results], axis=0
    )
    if _trace:
        return out, res
    return out


if __name__ == "__main__":
    rng = np.random.default_rng(0)
    coords = (rng.standard_normal((BATCH, N_ATOMS, 3)) * 5.0).astype(np.float32)
    atom_nc = rng.uniform(1.0, 50.0, (BATCH, NC2)).astype(np.float32)
    out = kernel(coords, atom_nc)
    print(out.shape, out.dtype)


# revision 23
# speedup vs baseline: 1.0189x; 1.0050x over previous
"""Trainium2 Bass kernel for nn_CoordsToNRF.

out[b, p] = atom_nc[b, p] * (AU2KCALMOLA / MAX_NRF) / ||coords[b, I[p]] - coords[b, J[p]]||^2

Strategy (pure data parallel over batch, 8 cores x 128 batches):
  - Layout: batch on partitions, pairs on the free dim.
  - Pair gather+subtract on the TensorEngine: per xyz component,
        D_c = CT_c.T @ S
    with S [atom, pairs] the +1/-1 tril selection matrix. Exact TWO-term
    fp16 split (~22 mantissa bits):  C = C0 + 2^-14*C1.
    The 2^-14 for the lo term is folded into a GLOBAL rescale so one S
    matrix serves both terms with no extra DMA or engine work:
        S' = S * 2^-11        (+-2^-11, exact fp16)
        CT_hi = C0 * 2^11     (exact exponent shift, |C0|*2^11 < 60000)
        CT_lo = C1 * 2^-3     (exact; subnormal flush loses < 3e-8)
    so  CT_hi.T @ S' + CT_lo.T @ S' = C0.T@S + 2^-14 * C1.T@S = D exactly.
  - Per 512-col group (one 3-bank PSUM tile): one ScalarE Square op over
    the 3 planes (scale folds 1/sqrt(K)), bf16 out.
  - Per 1024-col macro: DVE-only tail (adds at the bf16 2x rate, fast
    reciprocal, mul by fp16 atom_nc). GpSimd stays IDLE on purpose: the
    hardware activity limiter duty-caps the DVE+GpSimd pair at 50%, so
    any GpSimd work throttles the DVE ~3x (measured).
  - Head: input DMAs split across BOTH HWDGE rings -- ct then anc on the
    ScalarE ring, smat slices (small first) + outputs on the Sync ring.
    ct and smat[0:512] land concurrently ~9.5us (vs ~12us serialized).
    Matmuls run hi-term-first; dummy-matmul warmup bridges preamble ->
    first data so the PE HAM un-throttles by ~10us.
  - Tail: the last macro drains as one 512 chain plus two 256 chains
    (half-width final squares), keeping the after-last-matmul critical
    path short.
  - DMA halved vs f32: atom_nc uploaded fp16, output returned bf16.

Measured (NTFF): ~48.2us mean (47.5-49.5 typical) vs 54.4us baseline.
Breakdown: ~6.6us framework preamble, first real matmul ~10us (DMA
latency), steady state ~26us (DVE-paced at ~3.3us/macro), drain lag +
tail chains ~5us, last output DMA + postamble ~4us. Known run-to-run
variance +-1..2us from the free-running HAM window phase and SDMA
packet interleave.
"""

import sys

for _p in ("/opt/trn_rl_repo",):
    if _p not in sys.path:
        sys.path.insert(0, _p)

import numpy as np
import ml_dtypes
from contextlib import ExitStack

import concourse.bass as bass
import concourse.bacc as bacc
import concourse.tile as tile
from concourse import mybir
from concourse.bass_utils import run_bass_kernel_spmd
from concourse.dve_ops import RECIP_APPROX_FAST_CONSTS, RECIPROCAL_APPROX_FAST

F32 = mybir.dt.float32
F16 = mybir.dt.float16
BF16 = mybir.dt.bfloat16

N_ATOMS = 128
NC2 = N_ATOMS * (N_ATOMS - 1) // 2  # 8128
NPAD = 8192  # pad pairs to uniform 512-col groups; host drops the tail
BATCH = 1024
N_CORES = 8
BPC = BATCH // N_CORES  # 128 batches per core

AU2KCALMOLA = 627.5095 * 0.529177
MAX_NRF = 13036.0
K_CONST = AU2KCALMOLA / MAX_NRF
SQ_SCALE = float(1.0 / np.sqrt(K_CONST))  # fold K into the square
LO_SHIFT = 2.0 ** 14
HI_UP = 2.0 ** 11    # ct_hi scale (exact shift)
LO_DN = 2.0 ** -3    # ct_lo scale: 2^11 * 2^-14
S_DN = 2.0 ** -11    # smat scale

GROUP = 512           # one 3-bank PSUM tile per group
NG = NPAD // GROUP    # 16 groups
MACRO = 1024          # elementwise unit = 2 groups
NMAC = NPAD // MACRO  # 8 macros

# smat DMA slices (Sync HWDGE ring): small first slices so early groups
# start as soon as possible; ct/anc ride the ScalarE HWDGE ring in
# parallel. Keep the total DMA count moderate: only ~8 DMAHW semaphore
# lanes exist, and a DMA whose lane is still busy stalls its whole
# queue (16 small slices starved the PE for 6.9us).
SMAT_SLICES = [(0, 512), (512, 512), (1024, 1024), (2048, 2048),
               (4096, 2048), (6144, 2048)]
ANC_SLICES = [(0, 1024), (1024, 2048), (3072, 2048), (5120, 3072)]
# Dummy-matmul warmup: bridges the idle window between engine-preamble
# end (~7.2us) and the first smat slice landing so the PE p-state ramp
# is continuous and early groups run at full clock.
N_WARMUP_MM = 6

_I, _J = np.tril_indices(N_ATOMS, -1)


def _build_smat() -> np.ndarray:
    s = np.zeros((N_ATOMS, NPAD), dtype=np.float16)
    p = np.arange(NC2)
    s[_I, p] = S_DN
    s[_J, p] = -S_DN
    return s


def _build_program():
    nc = bacc.Bacc("TRN2", target_bir_lowering=False, debug=False)

    # ct: [atom, term, comp, batch] fp16, pre-transposed/split/scaled on host
    ct_d = nc.dram_tensor("ct", [N_ATOMS, 2 * 3 * BPC], F16, kind="ExternalInput")
    anc_d = nc.dram_tensor("anc", [BPC, NPAD], F16, kind="ExternalInput")
    smat_d = nc.dram_tensor("smat", [N_ATOMS, NPAD], F16, kind="ExternalInput")
    out_d = nc.dram_tensor("out", [BPC, NPAD], BF16, kind="ExternalOutput")

    rc = RECIP_APPROX_FAST_CONSTS

    with tile.TileContext(nc) as tc, ExitStack() as ctx:
        const = ctx.enter_context(tc.tile_pool(name="const", bufs=1))
        sqp = ctx.enter_context(tc.tile_pool(name="sqp", bufs=3))
        work = ctx.enter_context(tc.tile_pool(name="work", bufs=3))
        outp = ctx.enter_context(tc.tile_pool(name="outp", bufs=3))
        ps = ctx.enter_context(tc.tile_pool(name="ps", bufs=2, space="PSUM"))

        # ---- inputs ----
        # ct + anc on the ScalarE HWDGE ring (ScalarE is idle until the
        # first square); smat slices + outputs on the Sync ring. The two
        # rings transfer concurrently, halving the input-latency chain.
        ct_sb = const.tile([N_ATOMS, 2, 3, BPC], F16)
        nc.scalar.dma_start(
            ct_sb[:], ct_d[:, :].rearrange("a (t c b) -> a t c b", t=2, c=3)
        )
        smat_sb = {}
        for s0, w in SMAT_SLICES:
            st = const.tile([N_ATOMS, w], F16, tag=f"smat{s0}")
            nc.sync.dma_start(st[:], smat_d[:, s0:s0 + w])
            for g0 in range(s0, s0 + w, GROUP):
                smat_sb[g0] = (st, g0 - s0)
        anc_sb = []
        for s0, w in ANC_SLICES:
            at = const.tile([BPC, w], F16, tag=f"anc{s0}")
            nc.scalar.dma_start(at[:], anc_d[:, s0:s0 + w])
            anc_sb.append((s0, w, at))

        def anc_ap(c0, w):
            for s0, sw, at in anc_sb:
                if s0 <= c0 and c0 + w <= s0 + sw:
                    return at[:, c0 - s0:c0 - s0 + w]
            raise KeyError(c0)

        # ---- PE warmup. The HAM un-throttles the PE clock only after
        # ~3.4us of sustained activity, so start immediately after the
        # preamble barrier: a few tiny N=1 matmuls on the framework's
        # pre-initialized const tensors (no memset dependency), then
        # full-width dummy matmuls once warm_sb's memset lands. Sized so
        # warmup ends right as the first real smat/ct data arrives.
        warm_sb = const.tile([BPC, GROUP], F16, tag="warm")
        nc.gpsimd.memset(warm_sb[:], 0.0)
        warm_ps = ps.tile([BPC, GROUP], F32, tag="warm_ps")
        czero = nc.const_aps.tensor(0.0, [BPC, 1], F32)
        for _ in range(4):
            nc.tensor.matmul(
                warm_ps[0:1, 0:1], czero, czero, start=True, stop=True,
            )
        for _ in range(N_WARMUP_MM):
            nc.tensor.matmul(
                warm_ps[:, :], warm_sb[:, 0:BPC], warm_sb[:, :],
                start=True, stop=True,
            )

        def mm_group(g0, d_t):
            st, off = smat_sb[g0]
            for t in range(2):
                for c in range(3):
                    nc.tensor.matmul(
                        d_t[:, c, :],
                        ct_sb[:, t, c, :],
                        st[:, off:off + GROUP],
                        start=(t == 0),
                        stop=(t == 1),
                    )

        def recip(out_ap, in_ap):
            nc.vector._custom_dve(
                RECIPROCAL_APPROX_FAST, out=out_ap, in0=in_ap,
                s0=rc["s0"], s1=rc["s1"], imm2=rc["imm2"],
            )

        def square_group(sq_t, gi, d_t):
            # all 3 planes in one ScalarE op (flat AP saves ~110ns/op)
            nc.scalar.activation(
                sq_t[:, gi, :, :].rearrange("b c w -> b (c w)"),
                d_t[:, :, :].rearrange("b c w -> b (c w)"),
                mybir.ActivationFunctionType.Square,
                bias=0.0, scale=SQ_SCALE,
            )

        # ---- macros 0..6: steady-state pipeline ----
        for m in range(NMAC - 1):
            sq_t = sqp.tile([BPC, 2, 3, GROUP], BF16, tag="sq")
            for gi in range(2):
                g0 = m * MACRO + gi * GROUP
                d_t = ps.tile([BPC, 3, GROUP], F32, tag="d")
                mm_group(g0, d_t)
                square_group(sq_t, gi, d_t)
            # All-bf16 DVE tail: adds on the 2x path, fast reciprocal,
            # multiply by fp16 atom_nc.
            t01 = work.tile([BPC, MACRO], BF16, tag="t01")
            t01v = t01[:, :].rearrange("b (g w) -> b g w", g=2)
            nc.vector.tensor_add(t01v, sq_t[:, :, 0, :], sq_t[:, :, 1, :])
            r2 = work.tile([BPC, MACRO], BF16, tag="r2")
            r2v = r2[:, :].rearrange("b (g w) -> b g w", g=2)
            nc.vector.tensor_add(r2v, t01v, sq_t[:, :, 2, :])
            inv = work.tile([BPC, MACRO], BF16, tag="inv")
            # reciprocal_approx_fast with bf16 in/out APs: the DVE read
            # stage converts bf16->f32 lanes exactly (bits<<16), so the
            # bitwise seed trick is unaffected. Validated on HW: 3e-6.
            recip(inv[:, :], r2[:, :])
            o = outp.tile([BPC, MACRO], BF16)
            c0 = m * MACRO
            nc.vector.tensor_mul(o[:, :], inv[:, :], anc_ap(c0, MACRO))
            nc.sync.dma_start(out_d[:, c0:c0 + MACRO], o[:, :])
            if m == 0:
                # Bridge warmups: during the pipeline ramp the PE stalls
                # 1.5-2.7us here waiting for the first squares to free
                # the PSUM ring -- close to the HAM's 3.4us re-throttle
                # window (the occasional catastrophic-slow run). Keep the
                # PE array busy through the stall with throwaway matmuls.
                for _ in range(3):
                    nc.tensor.matmul(
                        warm_ps[:, :], warm_sb[:, 0:BPC], warm_sb[:, :],
                        start=True, stop=True,
                    )

        # ---- macro 7: short-chain drain ----
        # Group 14's 512 cols drain as one chain while group 15's matmuls
        # still run. Group 15 squares as two half-width ACTIVATEs feeding
        # two 256-col chains, so the after-last-matmul path is
        # sq(256) + short DVE chain + small DMA.
        sq_t = sqp.tile([BPC, 2, 3, GROUP], BF16, tag="sq")
        g14 = (NMAC - 1) * MACRO
        d14 = ps.tile([BPC, 3, GROUP], F32, tag="d")
        mm_group(g14, d14)
        nc.scalar.activation(
            sq_t[:, 0, :, :].rearrange("b c w -> b (c w)"),
            d14[:, :, :].rearrange("b c w -> b (c w)"),
            mybir.ActivationFunctionType.Square, bias=0.0, scale=SQ_SCALE,
        )
        g15 = g14 + GROUP
        d15 = ps.tile([BPC, 3, GROUP], F32, tag="d")
        mm_group(g15, d15)

        # chain A: group 14's 512 cols
        t01 = work.tile([BPC, MACRO], BF16, tag="t01")
        r2 = work.tile([BPC, MACRO], BF16, tag="r2")
        inv = work.tile([BPC, MACRO], BF16, tag="inv")
        o = outp.tile([BPC, MACRO], BF16)
        nc.vector.tensor_add(t01[:, 0:GROUP], sq_t[:, 0, 0, :], sq_t[:, 0, 1, :])
        nc.vector.tensor_add(r2[:, 0:GROUP], t01[:, 0:GROUP], sq_t[:, 0, 2, :])
        recip(inv[:, 0:GROUP], r2[:, 0:GROUP])
        nc.vector.tensor_mul(o[:, 0:GROUP], inv[:, 0:GROUP], anc_ap(g14, GROUP))
        nc.sync.dma_start(out_d[:, g14:g14 + GROUP], o[:, 0:GROUP])

        # group 15: two half-width square+chain pipelines
        H = GROUP // 2
        for h in range(2):
            hs = slice(h * H, (h + 1) * H)
            nc.scalar.activation(
                sq_t[:, 1, :, hs], d15[:, :, hs],
                mybir.ActivationFunctionType.Square, bias=0.0, scale=SQ_SCALE,
            )
            lo = GROUP + h * H
            c0 = g14 + lo
            nc.vector.tensor_add(
                t01[:, lo:lo + H], sq_t[:, 1, 0, hs], sq_t[:, 1, 1, hs],
            )
            nc.vector.tensor_add(
                r2[:, lo:lo + H], t01[:, lo:lo + H], sq_t[:, 1, 2, hs],
            )
            recip(inv[:, lo:lo + H], r2[:, lo:lo + H])
            nc.vector.tensor_mul(
                o[:, lo:lo + H], inv[:, lo:lo + H], anc_ap(c0, H)
            )
            nc.sync.dma_start(out_d[:, c0:c0 + H], o[:, lo:lo + H])

    nc.compile()
    return nc


_CACHED = None


def _get_program():
    global _CACHED
    if _CACHED is None:
        _CACHED = _build_program()
    return _CACHED


def _host_prep(coords, atom_nc):
    """Host-side sharding/layout only: fp16 hi/lo split (with the exact
    2^11 / 2^-3 exponent-shift scaling), transpose to [atom, term, comp,
    batch], fp16 atom_nc, padding to NPAD."""
    c32 = coords.astype(np.float32)
    c0 = c32.astype(np.float16)
    c1 = ((c32.astype(np.float64) - c0.astype(np.float64)) * LO_SHIFT).astype(
        np.float16
    )
    assert np.abs(c0.astype(np.float32)).max() * HI_UP < 60000.0
    hi = (c0.astype(np.float32) * HI_UP).astype(np.float16)
    lo = (c1.astype(np.float32) * LO_DN).astype(np.float16)
    # [B, A, 3] -> [cores, atom, term, comp, bpc]
    ct = np.empty((N_CORES, N_ATOMS, 2, 3, BPC), dtype=np.float16)
    for t, cc in enumerate((hi, lo)):
        r = cc.reshape(N_CORES, BPC, N_ATOMS, 3)
        ct[:, :, t, :, :] = r.transpose(0, 2, 3, 1)
    anc16 = np.ones((BATCH, NPAD), dtype=np.float16)
    anc16[:, :NC2] = atom_nc.astype(np.float16)
    return ct, anc16


def kernel(coords, atom_nc, _trace=False, _trace_kwargs=None):
    coords = np.ascontiguousarray(np.asarray(coords, dtype=np.float32))
    atom_nc = np.ascontiguousarray(np.asarray(atom_nc, dtype=np.float32))
    assert coords.shape == (BATCH, N_ATOMS, 3)
    assert atom_nc.shape == (BATCH, NC2)

    nc = _get_program()
    smat = _build_smat()
    ct, anc16 = _host_prep(coords, atom_nc)

    in_maps = []
    for core in range(N_CORES):
        b0 = core * BPC
        in_maps.append({
            "ct": ct[core].reshape(N_ATOMS, 2 * 3 * BPC),
            "anc": anc16[b0:b0 + BPC],
            "smat": smat,
        })

    kw = {}
    if _trace:
        kw["trace"] = True
        kw.update(_trace_kwargs or {})
    res = run_bass_kernel_spmd(nc, in_maps, core_ids=list(range(N_CORES)), **kw)
    out = np.concatenate(
        [r["out"][:, :NC2].astype(np.float32) for r in res.results], axis=0
    )
    if _trace:
        return out, res
    return out


if __name__ == "__main__":
    rng = np.random.default_rng(0)
    coords = (rng.standard_normal((BATCH, N_ATOMS, 3)) * 5.0).astype(np.float32)
    atom_nc = rng.uniform(1.0, 50.0, (BATCH, NC2)).astype(np.float32)
    out = kernel(coords, atom_nc)
    print(out.shape, out.dtype)
